# revision 88
# baseline (speedup 1.0000x reference)
"""KNN cross-sample attention on 8 Trainium2 NeuronCores (Bass/Tile).

Sharding: features (n=32) are split 4-per-core; the kNN mask / sample
reprs are computed once on host from the full batch and replicated.

Dataflow (v2c, ~125us HW vs 136us for the v1 baseline):
  * A@V flipped: exp(scores) em [keys, q] becomes the STATIONARY
    matmul operand (per 128-query block) and V the moving one, so the
    attention output lands QUERY-partitioned: out[q, dh].  V carries a
    33rd "ones" column per head, so the softmax denominator lands in
    the same PE instruction at out[q, 32] -- on the SAME partitions as
    the numerator.  This kills v1's dedicated den matmuls (-27us PE)
    AND the whole reciprocal-broadcast partition dance: normalize is a
    per-partition DVE op, no rb matmuls, no cross-partition moves.
  * each (j, q-block) A@V accumulation group runs its 4 key chunks
    back-to-back: interleaving open accumulation groups within one
    PSUM bank at the same PE tile position corrupts the accumulation
    (found the hard way; the per-head tile_position packing v1 used is
    the other safe pattern).
  * a cheap PE transpose restores the [inner, q] layout the output
    projection needs (f32: bf16 PSUM transpose tiles mis-size, and
    transpose outputs must land at PSUM partition 0).
  * masks are multiplicative on DVE; qkv psum->sbuf copies pinned to
    engines with slack (knobs).  Measured: all engines land at 60-75%
    of the ~125us wall; the PE runs throttled at ~1.2GHz effective, so
    further instruction-count cuts no longer move the wall.

Numpy fallback keeps the function correct if the device path fails.
"""

import os

import numpy as np

# ---------------- problem constants (self-contained) ----------------
B = 512
NF = 32
DIM = 256
H = 8
DH = 32
INNER = H * DH
K_NEIGHBORS = 16
SCALE = DH ** -0.5
N_CORES = 8
NF_PER_CORE = NF // N_CORES          # 4 features per core
MASK_NEG = -30.0

# units = (nf, half, key-chunk, head-pair) -> 4*2*4*2 = 64 per core.
# Per 8 consecutive units: first MASK_PE_NUM get the additive mask on
# the TensorEngine (pre-exp), next MASK_GP_NUM the multiplicative mask
# on GpSimd (post-exp), rest multiplicative on VectorE.
MASK_PE_NUM = int(os.environ.get("KNN_MASK_PE_NUM", "0"))
MASK_GP_NUM = int(os.environ.get("KNN_MASK_GP_NUM", "0"))
MASK_MOD = 8
AV_DELAY = int(os.environ.get("KNN_AV_DELAY", "2"))
# engine for qt/kt ([128,512]) and v ([128,256]) psum evacuation:
# "v"=vector, "s"=scalar, "g"=gpsimd
QK_COPY_ENG = os.environ.get("KNN_QK_COPY", "v")
V_COPY_ENG = os.environ.get("KNN_V_COPY", "s")
# normalize via 8x tensor_scalar (1) instead of one broadcast tensor_tensor (0)
OTN_TS = os.environ.get("KNN_OTN_TS", "0") == "1"
DEBUG_DUMPS = os.environ.get("KNN_DEBUG", "0") == "1"
PE_WARM = int(os.environ.get("KNN_PE_WARM", "0"))

LAST_EXEC_NS = None
_CACHED = {}


# ======================= device program =======================

def _build_bass():
    import concourse.bacc as bacc
    import concourse.mybir as mybir
    import concourse.tile as tile
    from concourse.alu_op_type import AluOpType
    from concourse.bass import broadcast_tensor_aps
    from concourse.bass_interp import get_hw_module
    from concourse.masks import make_identity

    f32 = mybir.dt.float32
    bf16 = mybir.dt.bfloat16
    EXP = mybir.ActivationFunctionType.Exp

    nc = bacc.Bacc(
        "TRN2", target_bir_lowering=False, debug=False,
        enable_asserts=False, num_devices=N_CORES,
    )

    def copy_to(tag, out, in_):
        if tag == "s":
            nc.scalar.copy(out, in_)
        elif tag == "g":
            nc.gpsimd.tensor_copy(out, in_)
        else:
            nc.vector.tensor_copy(out, in_)

    # ---- dram I/O (per core) ----
    xt_d = nc.dram_tensor("xt", [NF_PER_CORE, 2, 128, B], bf16, kind="ExternalInput")
    wq_d = nc.dram_tensor("wq", [2, 128, INNER], bf16, kind="ExternalInput")
    wk_d = nc.dram_tensor("wk", [2, 128, INNER], bf16, kind="ExternalInput")
    wv_d = nc.dram_tensor("wv", [2, 128, INNER], bf16, kind="ExternalInput")
    # W_out with rows reordered to the transposed-otn inner order:
    # row (ic, h2*64 + j*32 + d) = W_out[(4*ic + 2*h2 + j)*32 + d].
    wo_d = nc.dram_tensor("wo", [2, 128, DIM], bf16, kind="ExternalInput")
    bo_d = nc.dram_tensor("bo", [128, DIM], f32, kind="ExternalInput")
    ma_d = nc.dram_tensor("ma", [4, 128, B], bf16, kind="ExternalInput")
    mm_d = nc.dram_tensor("mm", [4, 128, 4 * B], bf16, kind="ExternalInput")
    # bf16 output: halves the final DMA drain; quantization (~4e-3) stays
    # well inside the 2e-2 gate (measured 3.7e-3 -> ~4.5e-3 total).
    y_d = nc.dram_tensor("y", [NF_PER_CORE, 4, 128, DIM], bf16, kind="ExternalOutput")
    dbg_d = nc.dram_tensor("dbg", [128, 2, 4, DH + 1], f32, kind="ExternalOutput")
    dbg2_d = nc.dram_tensor("dbg2", [128, 2 * B], bf16, kind="ExternalOutput")

    with tile.TileContext(nc) as tc:
        import contextlib
        with contextlib.ExitStack() as ctx:
            consts = ctx.enter_context(tc.tile_pool(name="consts", bufs=1))
            qkpool = ctx.enter_context(tc.tile_pool(name="qk", bufs=4))
            vpool = ctx.enter_context(tc.tile_pool(name="vp", bufs=2))
            empool = ctx.enter_context(tc.tile_pool(name="em", bufs=9))
            rcppool = ctx.enter_context(tc.tile_pool(name="rcp", bufs=3))
            otnpool = ctx.enter_context(tc.tile_pool(name="otn", bufs=5))
            ottpool = ctx.enter_context(tc.tile_pool(name="ott", bufs=2))
            outpool = ctx.enter_context(tc.tile_pool(name="outp", bufs=3))
            # PSUM: upool 2x2 banks + avq 2x1 + sm 2x1 = 8 banks
            upool = ctx.enter_context(
                tc.tile_pool(name="upool", bufs=2, space="PSUM"))
            avqpool = ctx.enter_context(
                tc.tile_pool(name="avqpool", bufs=2, space="PSUM"))
            smpool = ctx.enter_context(
                tc.tile_pool(name="smpool", bufs=2, space="PSUM"))

            # ---- load constants (compute-unblocking order) ----
            wq_sb = consts.tile([128, 2, INNER], bf16, tag="wq")
            wk_sb = consts.tile([128, 2, INNER], bf16, tag="wk")
            wv_sb = consts.tile([128, 2, INNER], bf16, tag="wv")
            xt_sb = consts.tile([128, NF_PER_CORE, 2, B], bf16, tag="xt")
            # issue order = first-use order: wq + xt[0] gate the first matmul
            for ch in range(2):
                nc.sync.dma_start(out=wq_sb[:, ch, :], in_=wq_d[ch])
            for ch in range(2):
                nc.sync.dma_start(out=xt_sb[:, 0, ch, :], in_=xt_d[0, ch])
            for sb, d in ((wk_sb, wk_d), (wv_sb, wv_d)):
                for ch in range(2):
                    nc.sync.dma_start(out=sb[:, ch, :], in_=d[ch])
            for nf in range(1, NF_PER_CORE):
                for ch in range(2):
                    nc.sync.dma_start(out=xt_sb[:, nf, ch, :], in_=xt_d[nf, ch])
            ma_sb = consts.tile([128, 4, B], bf16, tag="ma")
            mm_sb = consts.tile([128, 4, 4 * B], bf16, tag="mm")
            for c in range(4):
                nc.sync.dma_start(out=ma_sb[:, c, :], in_=ma_d[c])
                nc.sync.dma_start(out=mm_sb[:, c, :], in_=mm_d[c])
            wo_sb = consts.tile([128, 2, DIM], bf16, tag="wo")
            for ic in range(2):
                nc.sync.dma_start(out=wo_sb[:, ic, :], in_=wo_d[ic])
            bo_sb = consts.tile([128, DIM], f32, tag="bo")
            nc.sync.dma_start(out=bo_sb, in_=bo_d[:, :])
            ident = consts.tile([128, 128], bf16, tag="ident")
            make_identity(nc, ident)
            ident_f = consts.tile([128, 128], f32, tag="ident_f")
            make_identity(nc, ident_f)

            # Deferred emission queue (transpose + projection work traced
            # late so the PE stream never waits on the otn DVE chain).
            deferred = []

            def drain_one():
                if deferred:
                    deferred.pop(0)()

            qkv = {}

            def do_qkv(nf):
                """qkv projection for feature nf -> (qt[2], kt[2], v_ext)."""
                qt_half, kt_half = [], []
                for half in range(2):
                    hs = slice(128 * half, 128 * half + 128)
                    qt_ps = smpool.tile([128, B], f32, tag="sm", name="qt_ps")
                    for ch in range(2):
                        nc.tensor.matmul(
                            qt_ps, lhsT=wq_sb[:, ch, hs], rhs=xt_sb[:, nf, ch, :],
                            start=(ch == 0), stop=(ch == 1))
                    qt_sb = qkpool.tile([128, B], bf16, tag="qt", name="qt_sb")
                    copy_to(QK_COPY_ENG, qt_sb, qt_ps)
                    qt_half.append(qt_sb)

                    kt_ps = smpool.tile([128, B], f32, tag="sm", name="kt_ps")
                    for ch in range(2):
                        nc.tensor.matmul(
                            kt_ps, lhsT=wk_sb[:, ch, hs], rhs=xt_sb[:, nf, ch, :],
                            start=(ch == 0), stop=(ch == 1))
                    kt_sb = qkpool.tile([128, B], bf16, tag="kt", name="kt_sb")
                    copy_to(QK_COPY_ENG, kt_sb, kt_ps)
                    kt_half.append(kt_sb)

                # v extended with a ones column per head: [128, 4, 8, 33]
                v_sb = vpool.tile([128, 4, H, DH + 1], bf16, tag="v", name="v_sb")
                nc.vector.memset(v_sb[:, :, :, DH:DH + 1], 1.0)
                for bc in range(4):
                    bs = slice(128 * bc, 128 * bc + 128)
                    v_ps = smpool.tile([128, INNER], f32, tag="sm", name="v_ps")
                    for ch in range(2):
                        nc.tensor.matmul(
                            v_ps, lhsT=xt_sb[:, nf, ch, bs], rhs=wv_sb[:, ch, :],
                            start=(ch == 0), stop=(ch == 1))
                    copy_to(V_COPY_ENG, v_sb[:, bc, :, 0:DH],
                            v_ps.rearrange("p (h d) -> p h d", h=H))
                qkv[nf] = (qt_half, kt_half, v_sb)

            do_qkv(0)
            unit_idx = 0
            for nf in range(NF_PER_CORE):
                qt_half, kt_half, v_sb = qkv.pop(nf)
                otn_all = {}
                otnT_sb = ottpool.tile([128, 2, B], bf16, tag="otT",
                                       name="otnT_sb")
                for half in range(2):
                    qt_sb = qt_half[half]
                    kt_sb = kt_half[half]
                    # per head-pair: [q-part, head-in-pair, q-block, dh+den]
                    avq = [avqpool.tile([128, 2, 4, DH + 1], f32, tag="avq",
                                        name="avq_ps")
                           for _ in range(2)]
                    ems = {0: [], 1: []}

                    def emit_avq(h2, avq=avq, ems=ems, half=half, v_sb=v_sb):
                        # em as STATIONARY per 128-query block; V' (with
                        # ones col) moving -> out[q, dh+1].  Each (j, qb)
                        # accumulation group runs back-to-back over its 4
                        # key chunks: interleaving open accumulation groups
                        # within one PSUM bank at the same PE tile position
                        # corrupts the accumulation.
                        for j in range(2):
                            g = 4 * half + 2 * h2 + j
                            for qb in range(4):
                                qs = slice(B * j + 128 * qb,
                                           B * j + 128 * qb + 128)
                                for c in range(4):
                                    nc.tensor.matmul(
                                        avq[h2][:, j, qb, :],
                                        lhsT=ems[h2][c][:, qs],
                                        rhs=v_sb[:, c, g, :],
                                        start=(c == 0), stop=(c == 3),
                                        skip_group_check=True)

                    # h2-major unit order: head-pair 0's four key chunks
                    # first, then its A@V burst overlaps head-pair 1's
                    # scores instead of bunching all A@V at the half end.
                    # The burst for h2=0 is further delayed two units so its
                    # gate (exp+mask of chunk 3 on Scalar/DVE) resolves while
                    # the PE streams h2=1's scores; the h2=1 burst gets
                    # reserved deferred proj work as dependency-free filler.
                    for h2 in range(2):
                        for c in range(4):
                            cs = slice(128 * c, 128 * c + 128)
                            sel = unit_idx % MASK_MOD
                            # chunk-3 units (sel 3, 7 in h2-major order) mask
                            # additively on the PE: the A@V burst gate then
                            # skips the exp->DVE-mask cross-engine hop.
                            pe_mask = (sel % 4 == 3) or (sel < MASK_PE_NUM)
                            gp_mask = (not pe_mask) and sel < MASK_PE_NUM + MASK_GP_NUM
                            unit_idx += 1
                            u_ps = upool.tile([128, 2 * B], f32, tag="U", name="u_ps")
                            for j in range(2):
                                hh = 2 * h2 + j
                                ds = slice(32 * hh, 32 * hh + 32)
                                nc.tensor.matmul(
                                    u_ps[:, B * j:B * j + B],
                                    lhsT=kt_sb[ds, cs], rhs=qt_sb[ds, :],
                                    start=True, stop=not pe_mask,
                                    tile_position=(32 * hh, 0),
                                    skip_group_check=True)
                            if pe_mask:
                                for j in range(2):
                                    nc.tensor.matmul(
                                        u_ps[:, B * j:B * j + B],
                                        lhsT=ident, rhs=ma_sb[:, c, :],
                                        start=False, stop=True,
                                        skip_group_check=True)
                            em_t = empool.tile([128, 2 * B], bf16, tag="em", name="em_t")
                            nc.scalar.activation(em_t, u_ps, EXP)
                            if not pe_mask:
                                mm_slice = mm_sb[:, c, B * 2 * h2:B * 2 * h2 + 2 * B]
                                eng = nc.gpsimd if gp_mask else nc.vector
                                eng.tensor_tensor(
                                    em_t, em_t, mm_slice, op=AluOpType.mult)
                            if (DEBUG_DUMPS and nf == 0 and half == 0
                                    and c == 3 and h2 == 0):
                                nc.sync.dma_start(out=dbg2_d[:, :], in_=em_t)
                            ems[h2].append(em_t)
                            for _ in range(PE_WARM):
                                # dependency-free filler: keeps the PE busy
                                # through sem-wait gaps so the p-state ramp
                                # (3us continuous -> 2.4GHz) is not reset.
                                nc.tensor.ldweights(ident[:, 0:128])
                            if h2 == 0 and c in (1, 3):
                                drain_one()
                            if h2 == 1 and c == 1:
                                drain_one()
                                emit_avq(0)
                            if (nf + 1 < NF_PER_CORE and half == 0
                                    and h2 == 0 and c == 3):
                                do_qkv(nf + 1)
                    drain_one()
                    drain_one()
                    drain_one()
                    emit_avq(1)

                    # ---- tail (all DVE, inline): reciprocal of the fused
                    # denominators, then per-partition normalize.  Both live
                    # on the same q-partitions -- no broadcast needed.
                    # otn layout [q, qb, h2, j, d]: per-qb slice is a
                    # contiguous 128-col block, transposed whole (transpose
                    # outputs must land at PSUM partition 0).  f32 because
                    # the PE transpose output must match the lhsT dtype and
                    # bf16 PSUM storage is unreliable.
                    otn_t = otnpool.tile([128, 4, 2, 2, DH], f32, tag="otn",
                                         name="otn_t")
                    for h2 in range(2):
                        rcp_t = rcppool.tile([128, 2, 4, 1], f32, tag="rcp",
                                             name="rcp_t")
                        nc.vector.reciprocal_approx_fast(
                            rcp_t.rearrange("p a b c -> p (a b) c"),
                            avq[h2][:, :, :, DH:DH + 1].rearrange(
                                "p a b c -> p (a b) c"))
                        otn_w = otn_t[:, :, h2, :, :].rearrange(
                            "p qb j d -> p j qb d")
                        if OTN_TS:
                            for j in range(2):
                                for qb in range(4):
                                    nc.vector.tensor_scalar(
                                        otn_t[:, qb, h2, j, :],
                                        avq[h2][:, j, qb, 0:DH],
                                        rcp_t[:, j, qb, :], None,
                                        op0=AluOpType.mult)
                        else:
                            a_bc, r_bc = broadcast_tensor_aps(
                                avq[h2][:, :, :, 0:DH],
                                rcp_t)
                            nc.vector.tensor_tensor(
                                otn_w, a_bc, r_bc, op=AluOpType.mult)
                        if DEBUG_DUMPS and nf == 0 and half == 0 and h2 == 0:
                            dbg_sb = rcppool.tile([128, 2, 4, DH + 1], f32,
                                                  tag="dbg", name="dbg_sb")
                            nc.vector.tensor_copy(dbg_sb, avq[h2])
                            nc.sync.dma_start(out=dbg_d[:, :, :, :],
                                              in_=dbg_sb)
                    otn_all[half] = otn_t

                    # ---- transpose otn -> [inner, q] for the projection.
                    # per-half, deferred one unit: each half's transpose
                    # overlaps the next half's scores instead of bunching
                    # at the nf boundary.
                    def transp(ic=half, otn_all=otn_all, otnT_sb=otnT_sb):
                        otnT_ps = smpool.tile([128, B], f32, tag="sm",
                                              name="otnT_ps")
                        for qb in range(4):
                            nc.tensor.matmul(
                                otnT_ps[:, 128 * qb:128 * qb + 128],
                                lhsT=otn_all[ic][:, qb, :, :, :]
                                .rearrange("p a b c -> p (a b c)"),
                                rhs=ident_f,
                                is_transpose=True,
                                skip_group_check=True)
                        nc.scalar.copy(otnT_sb[:, ic, :], otnT_ps)
                    deferred.append(transp)

                # ---------- output projection (deferred one nf) ----------
                for bc in range(4):
                    def proj(nf=nf, bc=bc, otnT_sb=otnT_sb):
                        bs = slice(128 * bc, 128 * bc + 128)
                        pr_ps = smpool.tile([128, DIM], f32, tag="sm", name="pr_ps")
                        for ic in range(2):
                            nc.tensor.matmul(
                                pr_ps, lhsT=otnT_sb[:, ic, bs],
                                rhs=wo_sb[:, ic, :],
                                start=(ic == 0), stop=(ic == 1))
                        o_t = outpool.tile([128, DIM], bf16, tag="out", name="o_t")
                        nc.vector.tensor_add(o_t, pr_ps, bo_sb)
                        nc.sync.dma_start(out=y_d[nf, bc], in_=o_t)
                    deferred.append(proj)

            while deferred:
                drain_one()

    nc.compile()
    nc.m = get_hw_module(nc.m)
    return nc


# ======================= host side =======================

def _knn_mask(x, W_repr, b_repr):
    """chi[q, k] = 1 if k is among q's top-(K+1) cosine neighbours."""
    reprs = x.mean(axis=1) @ W_repr + b_repr
    normed = reprs / np.linalg.norm(reprs, axis=-1, keepdims=True)
    sim = normed @ normed.T
    k_actual = min(K_NEIGHBORS + 1, B)
    thresh = np.partition(sim, B - k_actual, axis=1)[:, B - k_actual]
    return sim >= thresh[:, None]                       # [B, B] bool


def _ensure_ntff_hook():
    """The agent image lacks antenv.axon_hooks; synthesize it from the
    boot module so run_bass_kernel_spmd(trace=True) can NTFF-profile."""
    import sys
    import types
    try:
        import antenv.axon_hooks  # noqa: F401
        return True
    except ImportError:
        pass
    try:
        from trn_agent_boot.trn_boot import _ntff_profile_via_ctypes
        hook = _ntff_profile_via_ctypes("/opt/axon/libaxon_pjrt.so")
    except Exception:
        return False
    if hook is None:
        return False
    import antenv
    mod = types.ModuleType("antenv.axon_hooks")
    mod.get_axon_ntff_profile_hook = lambda: hook
    mod.set_axon_ntff_profile_hook = lambda h: None
    antenv.axon_hooks = mod
    sys.modules["antenv.axon_hooks"] = mod
    return True


def _run_device(x, W_qkv, W_out, b_out, chi):
    global LAST_EXEC_NS
    import ml_dtypes
    from concourse.bass_utils import run_bass_kernel_spmd

    bf16 = ml_dtypes.bfloat16
    if "nc" not in _CACHED:
        _CACHED["nc"] = _build_bass()
    nc = _CACHED["nc"]

    # host-side input prep (cheap, all O(B*B) or O(x))
    chiT = chi.T                                         # [key, query]
    ma = (MASK_NEG * (~chiT).astype(np.float32)).reshape(4, 128, B).astype(bf16)
    mmul = np.broadcast_to(
        chiT.astype(np.float32).reshape(4, 128, 1, B), (4, 128, 4, B))
    mm = np.ascontiguousarray(mmul).reshape(4, 128, 4 * B).astype(bf16)

    xt = np.ascontiguousarray(x.transpose(1, 2, 0))      # [nf, din, b]
    xt = xt.reshape(NF, 2, 128, B).astype(bf16)          # chunk the din axis

    wq = np.ascontiguousarray(W_qkv[:, :INNER] * SCALE).reshape(2, 128, INNER).astype(bf16)
    wk = np.ascontiguousarray(W_qkv[:, INNER:2 * INNER]).reshape(2, 128, INNER).astype(bf16)
    wv = np.ascontiguousarray(W_qkv[:, 2 * INNER:]).reshape(2, 128, INNER).astype(bf16)
    # W_out rows reordered to the transposed-otn inner order:
    # row (ic, h2*64 + j*32 + d) = W_out[(4*ic + 2*h2 + j)*32 + d]
    wo = np.ascontiguousarray(W_out).reshape(2, 2, 2, DH, DIM)  # ic h2 j d
    wo = wo.reshape(2, 128, DIM).astype(bf16)
    bo = np.ascontiguousarray(np.broadcast_to(b_out, (128, DIM))).astype(np.float32)

    shared = {"wq": wq, "wk": wk, "wv": wv, "wo": wo, "bo": bo, "ma": ma, "mm": mm}
    in_maps = []
    for c in range(N_CORES):
        m = dict(shared)
        m["xt"] = np.ascontiguousarray(
            xt[c * NF_PER_CORE:(c + 1) * NF_PER_CORE])
        in_maps.append(m)

    trace = os.environ.get("KNN_TRACE", "0") == "1" and _ensure_ntff_hook()
    try:
        res = run_bass_kernel_spmd(
            nc, in_maps, core_ids=list(range(N_CORES)), trace=trace)
    except Exception:
        if not trace:
            raise
        res = run_bass_kernel_spmd(
            nc, in_maps, core_ids=list(range(N_CORES)), trace=False)
    if res.exec_time_ns is not None:
        LAST_EXEC_NS = res.exec_time_ns

    ys = np.stack([res.results[c]["y"] for c in range(N_CORES)])
    # ys: [core, nf_local, b_chunk, 128, dim] -> [b, nf, dim]
    out = ys.transpose(2, 3, 0, 1, 4).reshape(B, NF, DIM)
    return np.ascontiguousarray(out.astype(np.float32))


def _run_numpy(x, W_qkv, W_out, b_out, chi):
    qkv = x.reshape(B * NF, DIM) @ W_qkv
    qkv = qkv.reshape(B, NF, 3, H, DH)
    q = np.ascontiguousarray(qkv[:, :, 0].transpose(0, 2, 1, 3))
    k = np.ascontiguousarray(qkv[:, :, 1].transpose(0, 2, 1, 3))
    v = np.ascontiguousarray(qkv[:, :, 2].transpose(0, 2, 1, 3))
    sim = np.einsum("bhnd,Bhnd->nbhB", q, k).astype(np.float32) * SCALE
    masked = np.where(chi[None, :, None, :], sim, -np.inf)
    m = masked.max(axis=-1, keepdims=True)
    ex = np.where(chi[None, :, None, :], np.exp(sim - m), 0.0)
    attn = ex / ex.sum(axis=-1, keepdims=True)
    out = np.einsum("nbhB,Bhnd->bnhd", attn, v).reshape(B, NF, INNER)
    return ((out.reshape(B * NF, INNER) @ W_out + b_out)
            .reshape(B, NF, DIM).astype(np.float32))


def kernel(x, W_qkv, W_out, b_out, W_repr, b_repr):
    x = np.asarray(x, dtype=np.float32)
    W_qkv = np.asarray(W_qkv, dtype=np.float32)
    W_out = np.asarray(W_out, dtype=np.float32)
    b_out = np.asarray(b_out, dtype=np.float32)
    W_repr = np.asarray(W_repr, dtype=np.float32)
    b_repr = np.asarray(b_repr, dtype=np.float32)

    chi = _knn_mask(x, W_repr, b_repr)
    try:
        return _run_device(x, W_qkv, W_out, b_out, chi)
    except Exception:
        if os.environ.get("KNN_NO_FALLBACK", "0") == "1":
            raise
        return _run_numpy(x, W_qkv, W_out, b_out, chi)


# revision 89
# speedup vs baseline: 1.1198x; 1.1198x over previous
"""KNN cross-sample attention on 8 Trainium2 NeuronCores (Bass/Tile).

Sharding: features (n=32) are split 4-per-core; the kNN mask / sample
reprs are computed once on host from the full batch and replicated.

Dataflow (v2c, ~125us HW vs 136us for the v1 baseline):
  * A@V flipped: exp(scores) em [keys, q] becomes the STATIONARY
    matmul operand (per 128-query block) and V the moving one, so the
    attention output lands QUERY-partitioned: out[q, dh].  V carries a
    33rd "ones" column per head, so the softmax denominator lands in
    the same PE instruction at out[q, 32] -- on the SAME partitions as
    the numerator.  This kills v1's dedicated den matmuls (-27us PE)
    AND the whole reciprocal-broadcast partition dance: normalize is a
    per-partition DVE op, no rb matmuls, no cross-partition moves.
  * each (j, q-block) A@V accumulation group runs its 4 key chunks
    back-to-back: interleaving open accumulation groups within one
    PSUM bank at the same PE tile position corrupts the accumulation
    (found the hard way; the per-head tile_position packing v1 used is
    the other safe pattern).
  * a cheap PE transpose restores the [inner, q] layout the output
    projection needs (f32: bf16 PSUM transpose tiles mis-size, and
    transpose outputs must land at PSUM partition 0).
  * masks are multiplicative on DVE; qkv psum->sbuf copies pinned to
    engines with slack (knobs).  Measured: all engines land at 60-75%
    of the ~125us wall; the PE runs throttled at ~1.2GHz effective, so
    further instruction-count cuts no longer move the wall.

Numpy fallback keeps the function correct if the device path fails.
"""

import os

import numpy as np

# ---------------- problem constants (self-contained) ----------------
B = 512
NF = 32
DIM = 256
H = 8
DH = 32
INNER = H * DH
K_NEIGHBORS = 16
SCALE = DH ** -0.5
N_CORES = 8
NF_PER_CORE = NF // N_CORES          # 4 features per core
MASK_NEG = -30.0

# units = (nf, half, key-chunk, head-pair) -> 4*2*4*2 = 64 per core.
# Per 8 consecutive units: first MASK_PE_NUM get the additive mask on
# the TensorEngine (pre-exp), next MASK_GP_NUM the multiplicative mask
# on GpSimd (post-exp), rest multiplicative on VectorE.
MASK_PE_NUM = int(os.environ.get("KNN_MASK_PE_NUM", "0"))
MASK_GP_NUM = int(os.environ.get("KNN_MASK_GP_NUM", "0"))
MASK_MOD = 8
AV_DELAY = int(os.environ.get("KNN_AV_DELAY", "2"))
# engine for qt/kt ([128,512]) and v ([128,256]) psum evacuation:
# "v"=vector, "s"=scalar, "g"=gpsimd
QK_COPY_ENG = os.environ.get("KNN_QK_COPY", "v")
V_COPY_ENG = os.environ.get("KNN_V_COPY", "s")
# normalize via 8x tensor_scalar (1) instead of one broadcast tensor_tensor (0)
OTN_TS = os.environ.get("KNN_OTN_TS", "0") == "1"
DEBUG_DUMPS = os.environ.get("KNN_DEBUG", "0") == "1"
PE_WARM = int(os.environ.get("KNN_PE_WARM", "0"))

LAST_EXEC_NS = None
_CACHED = {}


# ======================= device program =======================

def _build_bass():
    import concourse.bacc as bacc
    import concourse.mybir as mybir
    import concourse.tile as tile
    from concourse.alu_op_type import AluOpType
    from concourse.bass import broadcast_tensor_aps
    from concourse.bass_interp import get_hw_module
    from concourse.masks import make_identity

    f32 = mybir.dt.float32
    bf16 = mybir.dt.bfloat16
    EXP = mybir.ActivationFunctionType.Exp

    nc = bacc.Bacc(
        "TRN2", target_bir_lowering=False, debug=False,
        enable_asserts=False, num_devices=N_CORES,
    )

    def copy_to(tag, out, in_):
        if tag == "s":
            nc.scalar.copy(out, in_)
        elif tag == "g":
            nc.gpsimd.tensor_copy(out, in_)
        else:
            nc.vector.tensor_copy(out, in_)

    # ---- dram I/O (per core) ----
    xt_d = nc.dram_tensor("xt", [NF_PER_CORE, 2, 128, B], bf16, kind="ExternalInput")
    wq_d = nc.dram_tensor("wq", [2, 128, INNER], bf16, kind="ExternalInput")
    wk_d = nc.dram_tensor("wk", [2, 128, INNER], bf16, kind="ExternalInput")
    wv_d = nc.dram_tensor("wv", [2, 128, INNER], bf16, kind="ExternalInput")
    # W_out with rows reordered to the transposed-otn inner order:
    # row (ic, h2*64 + j*32 + d) = W_out[(4*ic + 2*h2 + j)*32 + d].
    wo_d = nc.dram_tensor("wo", [2, 128, DIM], bf16, kind="ExternalInput")
    bo_d = nc.dram_tensor("bo", [128, DIM], f32, kind="ExternalInput")
    ma_d = nc.dram_tensor("ma", [4, 128, B], bf16, kind="ExternalInput")
    mm_d = nc.dram_tensor("mm", [4, 128, 4 * B], bf16, kind="ExternalInput")
    # bf16 output: halves the final DMA drain; quantization (~4e-3) stays
    # well inside the 2e-2 gate (measured 3.7e-3 -> ~4.5e-3 total).
    y_d = nc.dram_tensor("y", [NF_PER_CORE, 4, 128, DIM], bf16, kind="ExternalOutput")
    dbg_d = nc.dram_tensor("dbg", [128, 2, 4, DH + 1], f32, kind="ExternalOutput")
    dbg2_d = nc.dram_tensor("dbg2", [128, 2 * B], bf16, kind="ExternalOutput")

    with tile.TileContext(nc) as tc:
        import contextlib
        with contextlib.ExitStack() as ctx:
            consts = ctx.enter_context(tc.tile_pool(name="consts", bufs=1))
            qkpool = ctx.enter_context(tc.tile_pool(name="qk", bufs=4))
            vpool = ctx.enter_context(tc.tile_pool(name="vp", bufs=2))
            empool = ctx.enter_context(tc.tile_pool(name="em", bufs=9))
            rcppool = ctx.enter_context(tc.tile_pool(name="rcp", bufs=3))
            otnpool = ctx.enter_context(tc.tile_pool(name="otn", bufs=5))
            ottpool = ctx.enter_context(tc.tile_pool(name="ott", bufs=2))
            outpool = ctx.enter_context(tc.tile_pool(name="outp", bufs=3))
            # PSUM: upool 2x2 banks + avq 2x1 + sm 2x1 = 8 banks
            upool = ctx.enter_context(
                tc.tile_pool(name="upool", bufs=2, space="PSUM"))
            avqpool = ctx.enter_context(
                tc.tile_pool(name="avqpool", bufs=2, space="PSUM"))
            smpool = ctx.enter_context(
                tc.tile_pool(name="smpool", bufs=2, space="PSUM"))

            # ---- load constants (compute-unblocking order) ----
            wq_sb = consts.tile([128, 2, INNER], bf16, tag="wq")
            wk_sb = consts.tile([128, 2, INNER], bf16, tag="wk")
            wv_sb = consts.tile([128, 2, INNER], bf16, tag="wv")
            xt_sb = consts.tile([128, NF_PER_CORE, 2, B], bf16, tag="xt")
            # issue order = first-use order: wq + xt[0] gate the first matmul
            for ch in range(2):
                nc.sync.dma_start(out=wq_sb[:, ch, :], in_=wq_d[ch])
            for ch in range(2):
                nc.sync.dma_start(out=xt_sb[:, 0, ch, :], in_=xt_d[0, ch])
            for sb, d in ((wk_sb, wk_d), (wv_sb, wv_d)):
                for ch in range(2):
                    nc.sync.dma_start(out=sb[:, ch, :], in_=d[ch])
            for nf in range(1, NF_PER_CORE):
                for ch in range(2):
                    nc.sync.dma_start(out=xt_sb[:, nf, ch, :], in_=xt_d[nf, ch])
            ma_sb = consts.tile([128, 4, B], bf16, tag="ma")
            mm_sb = consts.tile([128, 4, 4 * B], bf16, tag="mm")
            for c in range(4):
                nc.sync.dma_start(out=ma_sb[:, c, :], in_=ma_d[c])
                nc.sync.dma_start(out=mm_sb[:, c, :], in_=mm_d[c])
            wo_sb = consts.tile([128, 2, DIM], bf16, tag="wo")
            for ic in range(2):
                nc.sync.dma_start(out=wo_sb[:, ic, :], in_=wo_d[ic])
            bo_sb = consts.tile([128, DIM], f32, tag="bo")
            nc.sync.dma_start(out=bo_sb, in_=bo_d[:, :])
            ident = consts.tile([128, 128], bf16, tag="ident")
            make_identity(nc, ident)
            ident_f = consts.tile([128, 128], f32, tag="ident_f")
            make_identity(nc, ident_f)

            # Deferred emission queue (transpose + projection work traced
            # late so the PE stream never waits on the otn DVE chain).
            deferred = []

            def drain_one():
                if deferred:
                    deferred.pop(0)()

            qkv = {}

            def do_qkv(nf):
                """qkv projection for feature nf -> (qt[2], kt[2], v_ext)."""
                qt_half, kt_half = [], []
                for half in range(2):
                    hs = slice(128 * half, 128 * half + 128)
                    qt_ps = smpool.tile([128, B], f32, tag="sm", name="qt_ps")
                    for ch in range(2):
                        nc.tensor.matmul(
                            qt_ps, lhsT=wq_sb[:, ch, hs], rhs=xt_sb[:, nf, ch, :],
                            start=(ch == 0), stop=(ch == 1))
                    qt_sb = qkpool.tile([128, B], bf16, tag="qt", name="qt_sb")
                    copy_to(QK_COPY_ENG, qt_sb, qt_ps)
                    qt_half.append(qt_sb)

                    kt_ps = smpool.tile([128, B], f32, tag="sm", name="kt_ps")
                    for ch in range(2):
                        nc.tensor.matmul(
                            kt_ps, lhsT=wk_sb[:, ch, hs], rhs=xt_sb[:, nf, ch, :],
                            start=(ch == 0), stop=(ch == 1))
                    kt_sb = qkpool.tile([128, B], bf16, tag="kt", name="kt_sb")
                    copy_to(QK_COPY_ENG, kt_sb, kt_ps)
                    kt_half.append(kt_sb)

                # v extended with a ones column per head: [128, 4, 8, 33]
                v_sb = vpool.tile([128, 4, H, DH + 1], bf16, tag="v", name="v_sb")
                nc.vector.memset(v_sb[:, :, :, DH:DH + 1], 1.0)
                for bc in range(4):
                    bs = slice(128 * bc, 128 * bc + 128)
                    v_ps = smpool.tile([128, INNER], f32, tag="sm", name="v_ps")
                    for ch in range(2):
                        nc.tensor.matmul(
                            v_ps, lhsT=xt_sb[:, nf, ch, bs], rhs=wv_sb[:, ch, :],
                            start=(ch == 0), stop=(ch == 1))
                    copy_to(V_COPY_ENG, v_sb[:, bc, :, 0:DH],
                            v_ps.rearrange("p (h d) -> p h d", h=H))
                qkv[nf] = (qt_half, kt_half, v_sb)

            do_qkv(0)
            unit_idx = 0
            for nf in range(NF_PER_CORE):
                qt_half, kt_half, v_sb = qkv.pop(nf)
                otn_all = {}
                otnT_sb = ottpool.tile([128, 2, B], bf16, tag="otT",
                                       name="otnT_sb")
                for half in range(2):
                    qt_sb = qt_half[half]
                    kt_sb = kt_half[half]
                    # per head-pair: [q-part, head-in-pair, q-block, dh+den]
                    avq = [avqpool.tile([128, 2, 4, DH + 1], f32, tag="avq",
                                        name="avq_ps")
                           for _ in range(2)]
                    ems = {0: [], 1: []}

                    def emit_avq(h2, avq=avq, ems=ems, half=half, v_sb=v_sb):
                        # em as STATIONARY per 128-query block; V' (with
                        # ones col) moving -> out[q, dh+1].  Each (j, qb)
                        # accumulation group runs back-to-back over its 4
                        # key chunks: interleaving open accumulation groups
                        # within one PSUM bank at the same PE tile position
                        # corrupts the accumulation.
                        for j in range(2):
                            g = 4 * half + 2 * h2 + j
                            for qb in range(4):
                                qs = slice(B * j + 128 * qb,
                                           B * j + 128 * qb + 128)
                                for c in range(4):
                                    nc.tensor.matmul(
                                        avq[h2][:, j, qb, :],
                                        lhsT=ems[h2][c][:, qs],
                                        rhs=v_sb[:, c, g, :],
                                        start=(c == 0), stop=(c == 3),
                                        skip_group_check=True)

                    # h2-major unit order: head-pair 0's four key chunks
                    # first, then its A@V burst overlaps head-pair 1's
                    # scores instead of bunching all A@V at the half end.
                    # The burst for h2=0 is further delayed two units so its
                    # gate (exp+mask of chunk 3 on Scalar/DVE) resolves while
                    # the PE streams h2=1's scores; the h2=1 burst gets
                    # reserved deferred proj work as dependency-free filler.
                    for h2 in range(2):
                        for c in range(4):
                            cs = slice(128 * c, 128 * c + 128)
                            sel = unit_idx % MASK_MOD
                            pe_mask = sel < MASK_PE_NUM
                            gp_mask = (not pe_mask) and sel < MASK_PE_NUM + MASK_GP_NUM
                            unit_idx += 1
                            u_ps = upool.tile([128, 2 * B], f32, tag="U", name="u_ps")
                            for j in range(2):
                                hh = 2 * h2 + j
                                ds = slice(32 * hh, 32 * hh + 32)
                                nc.tensor.matmul(
                                    u_ps[:, B * j:B * j + B],
                                    lhsT=kt_sb[ds, cs], rhs=qt_sb[ds, :],
                                    start=True, stop=not pe_mask,
                                    tile_position=(32 * hh, 0),
                                    skip_group_check=True)
                            if pe_mask:
                                for j in range(2):
                                    nc.tensor.matmul(
                                        u_ps[:, B * j:B * j + B],
                                        lhsT=ident, rhs=ma_sb[:, c, :],
                                        start=False, stop=True,
                                        skip_group_check=True)
                            em_t = empool.tile([128, 2 * B], bf16, tag="em", name="em_t")
                            nc.scalar.activation(em_t, u_ps, EXP)
                            if not pe_mask:
                                mm_slice = mm_sb[:, c, B * 2 * h2:B * 2 * h2 + 2 * B]
                                eng = nc.gpsimd if gp_mask else nc.vector
                                eng.tensor_tensor(
                                    em_t, em_t, mm_slice, op=AluOpType.mult)
                            if (DEBUG_DUMPS and nf == 0 and half == 0
                                    and c == 3 and h2 == 0):
                                nc.sync.dma_start(out=dbg2_d[:, :], in_=em_t)
                            ems[h2].append(em_t)
                            for _ in range(PE_WARM):
                                # dependency-free filler: keeps the PE busy
                                # through sem-wait gaps so the p-state ramp
                                # (3us continuous -> 2.4GHz) is not reset.
                                nc.tensor.ldweights(ident[:, 0:128])
                            if h2 == 0 and c in (1, 3):
                                drain_one()
                            if h2 == 1 and c == 1:
                                drain_one()
                                emit_avq(0)
                            if (nf + 1 < NF_PER_CORE and half == 0
                                    and h2 == 0 and c == 3):
                                do_qkv(nf + 1)
                    drain_one()
                    drain_one()
                    drain_one()
                    emit_avq(1)

                    # ---- tail (all DVE, inline): reciprocal of the fused
                    # denominators, then per-partition normalize.  Both live
                    # on the same q-partitions -- no broadcast needed.
                    # otn layout [q, qb, h2, j, d]: per-qb slice is a
                    # contiguous 128-col block, transposed whole (transpose
                    # outputs must land at PSUM partition 0).  f32 because
                    # the PE transpose output must match the lhsT dtype and
                    # bf16 PSUM storage is unreliable.
                    otn_t = otnpool.tile([128, 4, 2, 2, DH], f32, tag="otn",
                                         name="otn_t")
                    for h2 in range(2):
                        rcp_t = rcppool.tile([128, 2, 4, 1], f32, tag="rcp",
                                             name="rcp_t")
                        nc.vector.reciprocal_approx_fast(
                            rcp_t.rearrange("p a b c -> p (a b) c"),
                            avq[h2][:, :, :, DH:DH + 1].rearrange(
                                "p a b c -> p (a b) c"))
                        otn_w = otn_t[:, :, h2, :, :].rearrange(
                            "p qb j d -> p j qb d")
                        if OTN_TS:
                            for j in range(2):
                                for qb in range(4):
                                    nc.vector.tensor_scalar(
                                        otn_t[:, qb, h2, j, :],
                                        avq[h2][:, j, qb, 0:DH],
                                        rcp_t[:, j, qb, :], None,
                                        op0=AluOpType.mult)
                        else:
                            a_bc, r_bc = broadcast_tensor_aps(
                                avq[h2][:, :, :, 0:DH],
                                rcp_t)
                            nc.vector.tensor_tensor(
                                otn_w, a_bc, r_bc, op=AluOpType.mult)
                        if DEBUG_DUMPS and nf == 0 and half == 0 and h2 == 0:
                            dbg_sb = rcppool.tile([128, 2, 4, DH + 1], f32,
                                                  tag="dbg", name="dbg_sb")
                            nc.vector.tensor_copy(dbg_sb, avq[h2])
                            nc.sync.dma_start(out=dbg_d[:, :, :, :],
                                              in_=dbg_sb)
                    otn_all[half] = otn_t

                    # ---- transpose otn -> [inner, q] for the projection.
                    # per-half, deferred one unit: each half's transpose
                    # overlaps the next half's scores instead of bunching
                    # at the nf boundary.
                    def transp(ic=half, otn_all=otn_all, otnT_sb=otnT_sb):
                        otnT_ps = smpool.tile([128, B], f32, tag="sm",
                                              name="otnT_ps")
                        for qb in range(4):
                            nc.tensor.matmul(
                                otnT_ps[:, 128 * qb:128 * qb + 128],
                                lhsT=otn_all[ic][:, qb, :, :, :]
                                .rearrange("p a b c -> p (a b c)"),
                                rhs=ident_f,
                                is_transpose=True,
                                skip_group_check=True)
                        nc.scalar.copy(otnT_sb[:, ic, :], otnT_ps)
                    deferred.append(transp)

                # ---------- output projection (deferred one nf) ----------
                for bc in range(4):
                    def proj(nf=nf, bc=bc, otnT_sb=otnT_sb):
                        bs = slice(128 * bc, 128 * bc + 128)
                        pr_ps = smpool.tile([128, DIM], f32, tag="sm", name="pr_ps")
                        for ic in range(2):
                            nc.tensor.matmul(
                                pr_ps, lhsT=otnT_sb[:, ic, bs],
                                rhs=wo_sb[:, ic, :],
                                start=(ic == 0), stop=(ic == 1))
                        o_t = outpool.tile([128, DIM], bf16, tag="out", name="o_t")
                        nc.vector.tensor_add(o_t, pr_ps, bo_sb)
                        nc.sync.dma_start(out=y_d[nf, bc], in_=o_t)
                    deferred.append(proj)

            while deferred:
                drain_one()

    nc.compile()
    nc.m = get_hw_module(nc.m)
    return nc


# ======================= host side =======================

def _knn_mask(x, W_repr, b_repr):
    """chi[q, k] = 1 if k is among q's top-(K+1) cosine neighbours."""
    reprs = x.mean(axis=1) @ W_repr + b_repr
    normed = reprs / np.linalg.norm(reprs, axis=-1, keepdims=True)
    sim = normed @ normed.T
    k_actual = min(K_NEIGHBORS + 1, B)
    thresh = np.partition(sim, B - k_actual, axis=1)[:, B - k_actual]
    return sim >= thresh[:, None]                       # [B, B] bool


def _ensure_ntff_hook():
    """The agent image lacks antenv.axon_hooks; synthesize it from the
    boot module so run_bass_kernel_spmd(trace=True) can NTFF-profile."""
    import sys
    import types
    try:
        import antenv.axon_hooks  # noqa: F401
        return True
    except ImportError:
        pass
    try:
        from trn_agent_boot.trn_boot import _ntff_profile_via_ctypes
        hook = _ntff_profile_via_ctypes("/opt/axon/libaxon_pjrt.so")
    except Exception:
        return False
    if hook is None:
        return False
    import antenv
    mod = types.ModuleType("antenv.axon_hooks")
    mod.get_axon_ntff_profile_hook = lambda: hook
    mod.set_axon_ntff_profile_hook = lambda h: None
    antenv.axon_hooks = mod
    sys.modules["antenv.axon_hooks"] = mod
    return True


def _run_device(x, W_qkv, W_out, b_out, chi):
    global LAST_EXEC_NS
    import ml_dtypes
    from concourse.bass_utils import run_bass_kernel_spmd

    bf16 = ml_dtypes.bfloat16
    if "nc" not in _CACHED:
        _CACHED["nc"] = _build_bass()
    nc = _CACHED["nc"]

    # host-side input prep (cheap, all O(B*B) or O(x))
    chiT = chi.T                                         # [key, query]
    ma = (MASK_NEG * (~chiT).astype(np.float32)).reshape(4, 128, B).astype(bf16)
    mmul = np.broadcast_to(
        chiT.astype(np.float32).reshape(4, 128, 1, B), (4, 128, 4, B))
    mm = np.ascontiguousarray(mmul).reshape(4, 128, 4 * B).astype(bf16)

    xt = np.ascontiguousarray(x.transpose(1, 2, 0))      # [nf, din, b]
    xt = xt.reshape(NF, 2, 128, B).astype(bf16)          # chunk the din axis

    wq = np.ascontiguousarray(W_qkv[:, :INNER] * SCALE).reshape(2, 128, INNER).astype(bf16)
    wk = np.ascontiguousarray(W_qkv[:, INNER:2 * INNER]).reshape(2, 128, INNER).astype(bf16)
    wv = np.ascontiguousarray(W_qkv[:, 2 * INNER:]).reshape(2, 128, INNER).astype(bf16)
    # W_out rows reordered to the transposed-otn inner order:
    # row (ic, h2*64 + j*32 + d) = W_out[(4*ic + 2*h2 + j)*32 + d]
    wo = np.ascontiguousarray(W_out).reshape(2, 2, 2, DH, DIM)  # ic h2 j d
    wo = wo.reshape(2, 128, DIM).astype(bf16)
    bo = np.ascontiguousarray(np.broadcast_to(b_out, (128, DIM))).astype(np.float32)

    shared = {"wq": wq, "wk": wk, "wv": wv, "wo": wo, "bo": bo, "ma": ma, "mm": mm}
    in_maps = []
    for c in range(N_CORES):
        m = dict(shared)
        m["xt"] = np.ascontiguousarray(
            xt[c * NF_PER_CORE:(c + 1) * NF_PER_CORE])
        in_maps.append(m)

    trace = os.environ.get("KNN_TRACE", "0") == "1" and _ensure_ntff_hook()
    try:
        res = run_bass_kernel_spmd(
            nc, in_maps, core_ids=list(range(N_CORES)), trace=trace)
    except Exception:
        if not trace:
            raise
        res = run_bass_kernel_spmd(
            nc, in_maps, core_ids=list(range(N_CORES)), trace=False)
    if res.exec_time_ns is not None:
        LAST_EXEC_NS = res.exec_time_ns

    ys = np.stack([res.results[c]["y"] for c in range(N_CORES)])
    # ys: [core, nf_local, b_chunk, 128, dim] -> [b, nf, dim]
    out = ys.transpose(2, 3, 0, 1, 4).reshape(B, NF, DIM)
    return np.ascontiguousarray(out.astype(np.float32))


def _run_numpy(x, W_qkv, W_out, b_out, chi):
    qkv = x.reshape(B * NF, DIM) @ W_qkv
    qkv = qkv.reshape(B, NF, 3, H, DH)
    q = np.ascontiguousarray(qkv[:, :, 0].transpose(0, 2, 1, 3))
    k = np.ascontiguousarray(qkv[:, :, 1].transpose(0, 2, 1, 3))
    v = np.ascontiguousarray(qkv[:, :, 2].transpose(0, 2, 1, 3))
    sim = np.einsum("bhnd,Bhnd->nbhB", q, k).astype(np.float32) * SCALE
    masked = np.where(chi[None, :, None, :], sim, -np.inf)
    m = masked.max(axis=-1, keepdims=True)
    ex = np.where(chi[None, :, None, :], np.exp(sim - m), 0.0)
    attn = ex / ex.sum(axis=-1, keepdims=True)
    out = np.einsum("nbhB,Bhnd->bnhd", attn, v).reshape(B, NF, INNER)
    return ((out.reshape(B * NF, INNER) @ W_out + b_out)
            .reshape(B, NF, DIM).astype(np.float32))


def kernel(x, W_qkv, W_out, b_out, W_repr, b_repr):
    x = np.asarray(x, dtype=np.float32)
    W_qkv = np.asarray(W_qkv, dtype=np.float32)
    W_out = np.asarray(W_out, dtype=np.float32)
    b_out = np.asarray(b_out, dtype=np.float32)
    W_repr = np.asarray(W_repr, dtype=np.float32)
    b_repr = np.asarray(b_repr, dtype=np.float32)

    chi = _knn_mask(x, W_repr, b_repr)
    try:
        return _run_device(x, W_qkv, W_out, b_out, chi)
    except Exception:
        if os.environ.get("KNN_NO_FALLBACK", "0") == "1":
            raise
        return _run_numpy(x, W_qkv, W_out, b_out, chi)
